# revision 46
# baseline (speedup 1.0000x reference)
"""Distributed Trainium2 kernel for a 5-layer GCN (PyG GCNConv + BN + ReLU).

Strategy (8 NeuronCores, SPMD single graph):
  - Nodes are permuted (sorted by in-degree, dealt round-robin into 128-node
    tiles) and partitioned core-major: core c owns contiguous device rows.
  - Self-loops are materialized as ordinary edges with weight 1.
  - Per layer the aggregation input is a replicated DRAM table (AllGather of
    per-core shards).  Layers 1 and 5 are transform-first (table = h @ W, so
    the gathered width is min(cin, cout)); layers 2-4 aggregate-first.
  - Edges are processed in 128-edge chunks: dma_gather fetches table[src]
    rows edge-major into SBUF (4 SWDGE queues round-robin, deep buffering —
    this is the key throughput lever), and the block-sparse scatter matrix
    S (S[e, dst_local] = w_e) is built ON-CHIP by DVE from a tiny persistent
    (dst, w) stream via iota/is_equal/mult, then matmul'd on TensorE to
    accumulate the weighted segment-sum in PSUM.
  - GCN symmetric normalization is folded in: tables store dinv*h, and the
    dst-side dinv is applied per-tile after reduction.
  - BN statistics via per-partition accumulators + a tiny AllReduce, BN+ReLU
    applied as per-channel scale/bias on ScalarE.
  - Layer 5's transform (z5 = h4 @ W5) is fused into layer 4's epilogue in
    feature-major form, so no h4 table or wide AllGather exists.

All index/structure arrays (gather indices, (dst,w) streams, schedules) are
built on the host from the graph structure; degree/dinv is host-computed
graph normalization.  All O(E*C) and O(N*C*C) math runs on device.
"""

import math
import os
import sys

os.environ.setdefault("NEURON_SCRATCHPAD_PAGE_SIZE", "2048")  # MB

sys.path.insert(0, "/opt/trn_rl_repo")

import numpy as np
import ml_dtypes

import concourse.bass as bass
import concourse.mybir as mybir
import concourse.bacc as bacc
import concourse.tile as tile
from concourse import bass_utils

NC = 8
TILE = 128
NQ = int(os.environ.get("GCN_QUEUES", "4"))
GBUFS = int(os.environ.get("GCN_GBUFS", "6"))
F32 = mybir.dt.float32
BF16 = mybir.dt.bfloat16
I16 = mybir.dt.int16
EPS = 1e-5

# per-layer aggregation config for the 128->64->128->256->256->128 GCN:
#   mode: TF = transform-first (table already multiplied by W), AF = agg-first
#   aw:   gathered/aggregated width (== table width == gather elem)
#   tdt:  table dtype
LAYER_CFG = [
    ("TF", 64, ml_dtypes.bfloat16),
    ("AF", 64, ml_dtypes.bfloat16),
    ("AF", 128, ml_dtypes.bfloat16),
    ("AF", 256, ml_dtypes.bfloat16),
    ("TF", 128, ml_dtypes.bfloat16),
]
# gather elements must be >= 256B, so narrow bf16 tables are padded to 128
# cols (the pad columns carry garbage; matmuls only read the first aw cols)
def _ew(aw):
    return aw if aw >= TILE else TILE


# ----------------------------------------------------------------------------
# Host-side planning: permutation, chunk schedule, index/dstw images per core.
# ----------------------------------------------------------------------------

class Plan:
    pass


def _wrap16(tokens):
    """int16 token list [n*128] -> [128, n*8] image (token i at [i%16, i//16],
    replicated 8x down the partitions for the 8 Q7 cores)."""
    n = tokens.shape[0]
    img16 = np.ascontiguousarray(tokens.reshape(n // 16, 16).T)
    return np.tile(img16, (8, 1))


def build_plan(x, edge_index, edge_weight, widths):
    P = Plan()
    N, C0 = x.shape
    E = edge_index.shape[1]
    P.N, P.C0, P.E = N, C0, E
    ntiles = math.ceil(N / (NC * TILE)) * NC
    P.ntpc = ntiles // NC              # tiles per core
    P.npc = P.ntpc * TILE              # nodes per core
    P.Npad = ntiles * TILE
    P.HALF = (P.Npad // 2 + TILE - 1) // TILE * TILE
    assert P.HALF < 32768 and P.Npad - P.HALF < 32768

    src = np.asarray(edge_index[0], dtype=np.int64)
    dst = np.asarray(edge_index[1], dtype=np.int64)
    ew = np.asarray(edge_weight, dtype=np.float32)

    cnt = np.bincount(dst, minlength=N)
    deg = np.bincount(dst, weights=ew.astype(np.float64), minlength=N).astype(np.float32) + 1.0
    dinv = (1.0 / np.sqrt(deg)).astype(np.float32)

    order = np.argsort(-cnt, kind="stable")          # sorted orig ids, high degree first
    pos = np.arange(P.Npad)
    t, p = pos // TILE, pos % TILE
    dev_of_sorted = (t % NC) * P.npc + (t // NC) * TILE + p
    orig_of_dev = np.full(P.Npad, -1, dtype=np.int64)
    dev_of_orig = np.empty(N, dtype=np.int64)
    orig_of_dev[dev_of_sorted[:N]] = order
    dev_of_orig[order] = dev_of_sorted[:N]
    P.orig_of_dev, P.dev_of_orig, P.dinv = orig_of_dev, dev_of_orig, dinv

    # device-space edges + self loops (weight 1.0)
    sdev = np.concatenate([dev_of_orig[src], dev_of_orig])
    ddev = np.concatenate([dev_of_orig[dst], dev_of_orig])
    wall = np.concatenate([ew, np.ones(N, dtype=np.float32)])

    o = np.argsort(ddev, kind="stable")
    sdev, ddev, wall = sdev[o], ddev[o], wall[o]
    tile_of = ddev // TILE
    bounds = np.searchsorted(tile_of, np.arange(ntiles + 1))

    # per (core, tile-slot, half) edge groups; schedule shape shared by cores
    groups = {}
    nlo = np.ones(P.ntpc, dtype=np.int64)
    nhi = np.ones(P.ntpc, dtype=np.int64)
    for g in range(ntiles):
        c, k = g // P.ntpc, g % P.ntpc
        lo_, hi_ = bounds[g], bounds[g + 1]
        s_, d_, w_ = sdev[lo_:hi_], ddev[lo_:hi_] % TILE, wall[lo_:hi_]
        m = s_ < P.HALF
        for half, msk in ((0, m), (1, ~m)):
            ss, dd, ww = s_[msk], d_[msk], w_[msk]
            oo = np.argsort(dd, kind="stable")
            groups[(c, k, half)] = (ss[oo], dd[oo], ww[oo])
            n = max(1, math.ceil(len(ss) / TILE))
            if half == 0:
                nlo[k] = max(nlo[k], n)
            else:
                nhi[k] = max(nhi[k], n)
    P.nlo, P.nhi = nlo, nhi
    P.tot_chunks = int(nlo.sum() + nhi.sum())

    # flat idx images + (dst, w) chunk streams per core (identical shapes)
    idx_flats, dstw_flats = [], []
    for c in range(NC):
        idx_parts = []
        dstw = np.zeros((TILE, P.tot_chunks * 2), dtype=np.float32)
        qglob = 0
        for k in range(P.ntpc):
            for half, nsch in ((0, int(nlo[k])), (1, int(nhi[k]))):
                ss, dd, ww = groups[(c, k, half)]
                ntok = nsch * TILE
                tok = np.zeros(ntok, dtype=np.int16)
                base = P.HALF if half else 0
                tok[: len(ss)] = (ss - base).astype(np.int16)
                idx_parts.append(_wrap16(tok))
                ne = len(ss)
                for b in range(nsch):
                    sl = slice(b * TILE, min((b + 1) * TILE, ne))
                    cnt_b = max(0, sl.stop - sl.start)
                    if cnt_b > 0:
                        dstw[:cnt_b, 2 * qglob] = dd[sl]
                        dstw[:cnt_b, 2 * qglob + 1] = ww[sl]
                    qglob += 1
        idx_flats.append(np.ascontiguousarray(np.concatenate(idx_parts, axis=1)))
        dstw_flats.append(dstw)
    P.idx_flats, P.dstw_flats = idx_flats, dstw_flats
    P.idx_total = idx_flats[0].shape[1]

    # per-core maskdinv [128, ntpc] (0 at pad nodes), f32
    P.maskdinv = []
    for c in range(NC):
        md = np.zeros((TILE, P.ntpc), dtype=np.float32)
        for k in range(P.ntpc):
            devs = c * P.npc + k * TILE + np.arange(TILE)
            real = orig_of_dev[devs] >= 0
            md[real, k] = dinv[orig_of_dev[devs][real]]
        P.maskdinv.append(md)

    P.widths = widths
    P.layers = LAYER_CFG

    # h0T: per-core feature-major slice of dinv*x, [C0, npc] f32
    P.h0T = []
    for c in range(NC):
        img = np.zeros((TILE, P.npc), dtype=np.float32)
        devs = c * P.npc + np.arange(P.npc)
        real = orig_of_dev[devs] >= 0
        rows = orig_of_dev[devs][real]
        img[:C0, real] = (x[rows] * dinv[rows, None]).T
        P.h0T.append(img)
    return P


# ----------------------------------------------------------------------------
# Graph builder
# ----------------------------------------------------------------------------

def mdt(np_dtype):
    return BF16 if np_dtype == ml_dtypes.bfloat16 else F32


def build_graph(nc, P, widths):
    ntpc, npc, HALF, Npad = P.ntpc, P.npc, P.HALF, P.Npad
    NREAL = float(P.N)
    ablate = set(os.environ.get("GCN_ABLATE", "").split(","))
    nlayers = len(P.layers)

    # ---- external inputs -------------------------------------------------
    idx_in = nc.dram_tensor("idx", [TILE, P.idx_total], I16, kind="ExternalInput")
    dstb_in = nc.dram_tensor("dstb", [TILE, P.tot_chunks], BF16, kind="ExternalInput")
    wb_in = nc.dram_tensor("wb", [TILE, P.tot_chunks], BF16, kind="ExternalInput")
    md_in = nc.dram_tensor("maskdinv", [TILE, ntpc], F32, kind="ExternalInput")
    h0T_in = nc.dram_tensor("h0T", [TILE, npc], F32, kind="ExternalInput")
    iotab_in = nc.dram_tensor("iotab", [TILE, TILE], BF16, kind="ExternalInput")
    ident_in = nc.dram_tensor("ident", [TILE, TILE], BF16, kind="ExternalInput")
    identf_in = nc.dram_tensor("identf", [TILE, TILE], F32, kind="ExternalInput")
    w_ins, g_ins, b_ins = [], [], []
    dims = [P.C0] + list(widths)
    for li in range(nlayers):
        cin, cout = dims[li], dims[li + 1]
        kcs = math.ceil(cin / TILE)
        hvs = math.ceil(cout / TILE)
        w_ins.append(nc.dram_tensor(f"W{li}", [TILE, kcs * cout], F32, kind="ExternalInput"))
        g_ins.append(nc.dram_tensor(f"g{li}", [TILE, hvs], F32, kind="ExternalInput"))
        b_ins.append(nc.dram_tensor(f"bb{li}", [TILE, hvs], F32, kind="ExternalInput"))
    w5b_in = nc.dram_tensor("W5b", [TILE, 2 * TILE], BF16, kind="ExternalInput")
    out_t = nc.dram_tensor("out", [npc, widths[-1]], F32, kind="ExternalOutput")

    # ---- internal DRAM: aggregation tables + shards ----------------------
    tables, shards = [], []
    for li, (mode, aw, tdt_np) in enumerate(P.layers):
        tdt = mdt(tdt_np)
        tables.append(nc.dram_tensor(f"tab{li}", [Npad, _ew(aw)], tdt, kind="Internal",
                                     addr_space="Shared"))
        shards.append(nc.dram_tensor(f"shard{li}", [npc, _ew(aw)], tdt, kind="Internal"))
    ar_in = nc.dram_tensor("ar_in", [TILE, 4], F32, kind="Internal")
    ar_out = nc.dram_tensor("ar_out", [TILE, 4], F32, kind="Internal", addr_space="Shared")

    from contextlib import ExitStack
    with tile.TileContext(nc) as tc, ExitStack() as es:
        pool = es.enter_context(tc.tile_pool(name="persist", bufs=1))
        gpool = es.enter_context(tc.tile_pool(name="gather", bufs=GBUFS))
        spool = es.enter_context(tc.tile_pool(name="sblk", bufs=3))
        epool = es.enter_context(tc.tile_pool(name="epi", bufs=3))
        ppool = es.enter_context(tc.tile_pool(name="psum", bufs=4, space="PSUM"))
        tpool = es.enter_context(tc.tile_pool(name="tpsum", bufs=2, space="PSUM"))
        ypool = es.enter_context(tc.tile_pool(name="ypsum", bufs=2, space="PSUM"))
        ybpool = es.enter_context(tc.tile_pool(name="ybuf", bufs=1))

        # persistent loads
        idx_sb = pool.tile([TILE, P.idx_total], I16)
        nc.sync.dma_start(idx_sb[:], idx_in[:, :])
        dstb_sb = pool.tile([TILE, P.tot_chunks], BF16)
        nc.sync.dma_start(dstb_sb[:], dstb_in[:, :])
        wb_sb = pool.tile([TILE, P.tot_chunks], BF16)
        nc.sync.dma_start(wb_sb[:], wb_in[:, :])
        md_sb = pool.tile([TILE, ntpc], F32)
        nc.sync.dma_start(md_sb[:], md_in[:, :])
        iotab_sb = pool.tile([TILE, TILE], BF16)
        nc.sync.dma_start(iotab_sb[:], iotab_in[:, :])
        ident_sb = pool.tile([TILE, TILE], BF16)
        nc.sync.dma_start(ident_sb[:], ident_in[:, :])
        identf_sb = pool.tile([TILE, TILE], F32)
        nc.sync.dma_start(identf_sb[:], identf_in[:, :])
        w_sb, g_sb, b_sb = [], [], []
        for li in range(nlayers):
            cin, cout = dims[li], dims[li + 1]
            kcs = math.ceil(cin / TILE)
            hvs = math.ceil(cout / TILE)
            wt = pool.tile([TILE, kcs * cout], F32, name=f"w{li}sb")
            nc.sync.dma_start(wt[:], w_ins[li][:, :])
            w_sb.append(wt)
            gt_ = pool.tile([TILE, hvs], F32, name=f"g{li}sb")
            nc.sync.dma_start(gt_[:], g_ins[li][:, :])
            g_sb.append(gt_)
            bt = pool.tile([TILE, hvs], F32, name=f"b{li}sb")
            nc.sync.dma_start(bt[:], b_ins[li][:, :])
            b_sb.append(bt)
        w5b_sb = pool.tile([TILE, 2 * TILE], BF16)
        nc.sync.dma_start(w5b_sb[:], w5b_in[:, :])

        # idx / chunk offsets per (k, half)
        idx_off, chk_off = {}, {}
        io = co = 0
        for k in range(ntpc):
            for half, n in ((0, int(P.nlo[k])), (1, int(P.nhi[k]))):
                idx_off[(k, half)] = io
                chk_off[(k, half)] = co
                io += n * TILE // 16
                co += n

        def allgather(shard_t, table_t):
            if "noag" not in ablate:
                nc.gpsimd.collective_compute(
                    "AllGather", mybir.AluOpType.bypass,
                    replica_groups=[list(range(NC))],
                    ins=[shard_t.ap()], outs=[table_t.ap()])
            else:
                nc.sync.dma_start(table_t[0:npc, :], shard_t[0:npc, :])

        # ---- stage 0: z1 = (dinv*x) @ W1 for own rows, AllGather ---------
        for k in range(ntpc):
            h0t = gpool.tile([TILE, TILE], F32, name="h0t", tag="h0t")
            nc.sync.dma_start(h0t[:], h0T_in[:, k * TILE:(k + 1) * TILE])
            pz = ypool.tile([TILE, TILE], F32, name="py", tag="py")
            nc.tensor.matmul(pz[:, 0:64], h0t[:, :],
                             w_sb[0][:, 0:64], start=True, stop=True)
            stg = epool.tile([TILE, 64], BF16, name="z1s", tag="stage")
            nc.scalar.copy(stg[:, :], pz[:, 0:64])
            nc.sync.dma_start(shards[0][k * TILE:(k + 1) * TILE, 0:64], stg[:, :])
        allgather(shards[0], tables[0])

        gq = [0]  # round-robin SWDGE queue for gathers

        for li, (mode, aw, tdt_np) in enumerate(P.layers):
            tdt = mdt(tdt_np)
            table = tables[li]
            cout = widths[li]
            cin = dims[li]
            kcs = math.ceil(aw / TILE)       # feature chunks of aggregated z
            hvs = math.ceil(cout / TILE)
            last = li == nlayers - 1
            # next-table dtype/width for staging (AF tables are pre-scaled by
            # dinv; z5 staging is handled specially in the li==3 branch)
            if not last:
                tdt_next = mdt(P.layers[li + 1][2])
                aw_next = P.layers[li + 1][1]

            strip_sum = epool.tile([TILE, ntpc * hvs], F32, name=f"ssum{li}", bufs=1)
            strip_sq = epool.tile([TILE, ntpc * hvs], F32, name=f"ssq{li}", bufs=1)
            # f32 parking where the consumer is f32 (final out)
            ydt = F32 if li == nlayers - 1 else BF16
            ybuf = ybpool.tile([TILE, ntpc * hvs * TILE], ydt, name=f"ybuf{li}",
                               tag="ybuf")

            for k in range(ntpc):
                psz = ppool.tile([TILE, aw], F32, name=f"psz{li}", tag="psz")
                nch_tot = int(P.nlo[k]) + int(P.nhi[k])
                qglob = 0
                ew = _ew(aw)
                for half in (0, 1):
                    n = int(P.nlo[k]) if half == 0 else int(P.nhi[k])
                    ntok = n * TILE
                    ioff = idx_off[(k, half)]
                    coff = chk_off[(k, half)]
                    # gather table[src] rows edge-major
                    gt = gpool.tile([TILE, n * ew], tdt, name=f"gt{li}", tag="gath")
                    base_ap = table[0:HALF, :] if half == 0 else table[HALF:Npad, :]
                    if "nogather" in ablate:
                        nc.sync.dma_start(
                            gt[:], table[0:ntok, :].rearrange(
                                "(a b) e -> a (b e)", a=TILE))
                    else:
                        nc.gpsimd.dma_gather(
                            gt.rearrange("p (q e) -> p q e", e=ew),
                            base_ap,
                            idx_sb[:, ioff: ioff + ntok // 16],
                            ntok, ntok, ew,
                            single_packet=False,
                            queue_num=gq[0] % NQ,
                        )
                        gq[0] += 1
                    # build S on-chip: S[p, c] = (c == dst[p]) * w[p].
                    # DVE side uses tensor_tensor only (1-port mode):
                    # tensor_scalar/copy can enter 2-port mode and lock GpSimd
                    # out of the SBUF port it needs for SWDGE gather
                    # descriptor generation.  The one-hot compare always runs
                    # in bf16 (exact for ints <= 128, 2x DVE rate); for f32
                    # tables the w-scale/upcast runs per-chunk on ACT (never
                    # contends) via Copy with a per-partition scale.
                    st = spool.tile([TILE, n * TILE], BF16, name=f"st{li}", tag="sblk")
                    if "nosb" not in ablate or li == 0:
                        nc.vector.tensor_tensor(
                            st[:].rearrange("p (q c) -> p q c", c=TILE),
                            iotab_sb[:, :].rearrange("p (b c) -> p b c", b=1)
                                .broadcast_to([TILE, n, TILE]),
                            dstb_sb[:, coff:coff + n]
                                .rearrange("p (q b) -> p q b", b=1)
                                .broadcast_to([TILE, n, TILE]),
                            mybir.AluOpType.is_equal)
                        nc.vector.tensor_tensor(
                            st[:].rearrange("p (q c) -> p q c", c=TILE),
                            st[:].rearrange("p (q c) -> p q c", c=TILE),
                            wb_sb[:, coff:coff + n]
                                .rearrange("p (q b) -> p q b", b=1)
                                .broadcast_to([TILE, n, TILE]),
                            mybir.AluOpType.mult)
                    for b in range(n):
                        if "nope" in ablate and not (qglob == 0 or qglob == nch_tot - 1):
                            qglob += 1
                            continue
                        nc.tensor.matmul(
                            psz[:, :],
                            st[:, b * TILE:(b + 1) * TILE],
                            gt[:, b * ew:b * ew + aw],
                            start=(qglob == 0),
                            stop=True if "nope" in ablate else (qglob == nch_tot - 1),
                        )
                        qglob += 1

                # epilogue: z = psz * maskdinv (on ACT: keeps the psz PSUM
                # slot release off the loaded DVE queue)
                z_sb = epool.tile([TILE, aw], F32, name=f"z{li}", tag="z")
                nc.scalar.activation(
                    z_sb[:], psz[:], mybir.ActivationFunctionType.Copy,
                    scale=md_sb[:, k:k + 1])
                ys = []          # feature-major y tiles (PSUM or SBUF), w/ width
                if mode == "TF":
                    # already transformed: y = z^T directly
                    for kc in range(kcs):
                        w = min(TILE, aw - kc * TILE)
                        pt = tpool.tile([TILE, TILE], F32, name=f"pzt{li}", tag="tp")
                        nc.tensor.transpose(pt[:w, :TILE],
                                            z_sb[:, kc * TILE: kc * TILE + w],
                                            identf_sb[:TILE, :TILE])
                        ys.append((pt, w))
                else:
                    zT = []
                    for kc in range(kcs):
                        w = min(TILE, aw - kc * TILE)
                        pt = tpool.tile([TILE, TILE], F32, name=f"pzt{li}", tag="tp")
                        nc.tensor.transpose(pt[:w, :TILE],
                                            z_sb[:, kc * TILE: kc * TILE + w],
                                            identf_sb[:TILE, :TILE])
                        zt = epool.tile([TILE, TILE], F32, name=f"zt{li}", tag="zt")
                        nc.scalar.copy(zt[:w, :], pt[:w, :])
                        zT.append((zt, w))
                    for h in range(hvs):
                        hw = min(TILE, cout - h * TILE)
                        py = ypool.tile([TILE, TILE], F32, name=f"py{li}", tag="py")
                        for kc in range(kcs):
                            zt, w = zT[kc]
                            nc.tensor.matmul(
                                py[:hw, :TILE],
                                w_sb[li][:w, kc * cout + h * TILE: kc * cout + h * TILE + hw],
                                zt[:w, :TILE],
                                start=(kc == 0), stop=(kc == kcs - 1),
                            )
                        ys.append((py, hw))

                for h, (py, hw) in enumerate(ys):
                    col = k * hvs + h
                    # park + stats on ACT (never contends with GpSimd/SWDGE)
                    nc.scalar.activation(
                        ybuf[:hw, col * TILE:(col + 1) * TILE], py[:hw, :TILE],
                        mybir.ActivationFunctionType.Copy,
                        accum_out=strip_sum[:hw, col:col + 1])
                    sq = epool.tile([TILE, TILE], F32, name=f"sq{li}", tag="sq")
                    nc.scalar.activation(sq[:hw, :], py[:hw, :TILE],
                                         mybir.ActivationFunctionType.Square,
                                         accum_out=strip_sq[:hw, col:col + 1])

            # ---- BN stats: reduce strips, AllReduce, scale/bias ----
            pack = epool.tile([TILE, 4], F32, name=f"pack{li}", tag="pack")
            nc.gpsimd.memset(pack[:], 0.0)
            for h in range(hvs):
                hw = min(TILE, cout - h * TILE)
                nc.vector.tensor_reduce(
                    pack[:hw, h:h + 1],
                    strip_sum[:hw, h::hvs] if hvs > 1 else strip_sum[:hw, :],
                    mybir.AxisListType.X, mybir.AluOpType.add)
                nc.vector.tensor_reduce(
                    pack[:hw, 2 + h:3 + h],
                    strip_sq[:hw, h::hvs] if hvs > 1 else strip_sq[:hw, :],
                    mybir.AxisListType.X, mybir.AluOpType.add)
            nc.sync.dma_start(ar_in[:, :], pack[:])
            if "noag" not in ablate:
                nc.gpsimd.collective_compute(
                    "AllReduce", mybir.AluOpType.add,
                    replica_groups=[list(range(NC))],
                    ins=[ar_in.ap()], outs=[ar_out.ap()])
            arr = epool.tile([TILE, 4], F32, name=f"arr{li}", tag="arr")
            nc.sync.dma_start(arr[:], (ar_out if "noag" not in ablate else ar_in)[:, :])
            mvec = epool.tile([TILE, 2], F32, name=f"m{li}", tag="mv")
            nc.vector.tensor_scalar(mvec[:, 0:2], arr[:, 0:2], 1.0 / NREAL, None,
                                    mybir.AluOpType.mult)
            vvec = epool.tile([TILE, 2], F32, name=f"v{li}", tag="vv")
            nc.vector.tensor_scalar(vvec[:, 0:2], arr[:, 2:4], 1.0 / NREAL, None,
                                    mybir.AluOpType.mult)
            msq = epool.tile([TILE, 2], F32, name=f"msq{li}", tag="msq")
            nc.vector.tensor_tensor(msq[:, :], mvec[:, :], mvec[:, :], mybir.AluOpType.mult)
            nc.vector.tensor_tensor(vvec[:, :], vvec[:, :], msq[:, :], mybir.AluOpType.subtract)
            nc.vector.tensor_scalar(vvec[:, :], vvec[:, :], EPS, None, mybir.AluOpType.add)
            sqr = epool.tile([TILE, 2], F32, name=f"sqr{li}", tag="sqr")
            nc.scalar.activation(sqr[:, :], vvec[:, :], mybir.ActivationFunctionType.Sqrt)
            rin = epool.tile([TILE, 2], F32, name=f"rin{li}", tag="rin")
            nc.vector.reciprocal(rin[:, :], sqr[:, :])
            scl = epool.tile([TILE, 2], F32, name=f"scl{li}", tag="scl")
            nc.vector.tensor_tensor(scl[:, 0:hvs], rin[:, 0:hvs], g_sb[li][:, 0:hvs],
                                    mybir.AluOpType.mult)
            bia = epool.tile([TILE, 2], F32, name=f"bia{li}", tag="bia")
            nc.vector.tensor_tensor(bia[:, 0:hvs], mvec[:, 0:hvs], scl[:, 0:hvs],
                                    mybir.AluOpType.mult)
            nc.vector.tensor_tensor(bia[:, 0:hvs], b_sb[li][:, 0:hvs], bia[:, 0:hvs],
                                    mybir.AluOpType.subtract)

            # ---- BN apply (feature-major) + produce next table / output ----
            for k in range(ntpc):
                bns = []
                for h in range(hvs):
                    hw = min(TILE, cout - h * TILE)
                    col = k * hvs + h
                    bn = epool.tile([TILE, TILE], F32 if last else BF16,
                                    name=f"bn{li}", tag="bn")
                    if not last:
                        nc.scalar.activation(
                            bn[:hw, :], ybuf[:hw, col * TILE:(col + 1) * TILE],
                            mybir.ActivationFunctionType.Relu,
                            bias=bia[:hw, h:h + 1], scale=scl[:hw, h:h + 1])
                    else:
                        nc.scalar.activation(
                            bn[:hw, :], ybuf[:hw, col * TILE:(col + 1) * TILE],
                            mybir.ActivationFunctionType.Identity,
                            bias=bia[:hw, h:h + 1], scale=scl[:hw, h:h + 1])
                    bns.append((bn, hw))

                if li == 3:
                    # fuse z5 = h4 @ W5 (feature-major h4 = bn tiles), dinv-scale
                    pz5 = ypool.tile([TILE, TILE], F32, name="pz5", tag="py")
                    for h, (bn, hw) in enumerate(bns):
                        nc.tensor.matmul(
                            pz5[:, :TILE], bn[:hw, :TILE],
                            w5b_sb[:hw, h * TILE:(h + 1) * TILE],
                            start=(h == 0), stop=(h == len(bns) - 1))
                    stage = epool.tile([TILE, TILE], BF16, name=f"stg{li}", tag="stage")
                    nc.vector.tensor_tensor(
                        stage[:, :], pz5[:, :],
                        md_sb[:, k:k + 1].broadcast_to([TILE, TILE]),
                        mybir.AluOpType.mult)
                    nc.sync.dma_start(shards[4][k * TILE:(k + 1) * TILE, :],
                                      stage[:, :])
                elif last:
                    stage = epool.tile([TILE, cout], F32, name=f"stg{li}", tag="stage")
                    for h, (bn, hw) in enumerate(bns):
                        pt2 = tpool.tile([TILE, TILE], F32, name=f"pt2{li}", tag="tp")
                        nc.tensor.transpose(pt2[:TILE, :hw], bn[:hw, :TILE],
                                            identf_sb[:hw, :hw])
                        nc.scalar.copy(stage[:, h * TILE: h * TILE + hw],
                                       pt2[:, :hw])
                    nc.sync.dma_start(out_t[k * TILE:(k + 1) * TILE, :], stage[:, :])
                else:
                    # next AF table: transpose to node-major, pre-scale by dinv
                    stage = epool.tile([TILE, aw_next], tdt_next,
                                       name=f"stg{li}", tag="stage")
                    idm = identf_sb if tdt_next == F32 else ident_sb
                    for h, (bn, hw) in enumerate(bns):
                        pt2 = tpool.tile([TILE, TILE], F32 if tdt_next == F32 else BF16,
                                         name=f"pt2{li}", tag="tp")
                        nc.tensor.transpose(pt2[:TILE, :hw], bn[:hw, :TILE],
                                            idm[:hw, :hw])
                        nc.vector.tensor_tensor(
                            stage[:, h * TILE: h * TILE + hw], pt2[:, :hw],
                            md_sb[:, k:k + 1].broadcast_to([TILE, hw]),
                            mybir.AluOpType.mult)
                    nc.sync.dma_start(
                        shards[li + 1][k * TILE:(k + 1) * TILE, 0:aw_next],
                        stage[:, :aw_next])

            if li == 3:
                allgather(shards[4], tables[4])
            elif not last:
                allgather(shards[li + 1], tables[li + 1])

    return nc


# ----------------------------------------------------------------------------
# Entry point
# ----------------------------------------------------------------------------

def kernel(**inputs):
    x = np.asarray(inputs["x"], dtype=np.float32)
    edge_index = np.asarray(inputs["edge_index"])
    edge_weight = np.asarray(inputs["edge_weight"], dtype=np.float32)
    widths = []
    i = 1
    while f"W{i}" in inputs:
        widths.append(np.asarray(inputs[f"W{i}"]).shape[1])
        i += 1

    P = build_plan(x, edge_index, edge_weight, widths)

    weights = [np.asarray(inputs[f"W{i+1}"], dtype=np.float32) for i in range(len(widths))]
    gammas = [np.asarray(inputs[f"g{i+1}"], dtype=np.float32) for i in range(len(widths))]
    betas = [np.asarray(inputs[f"bb{i+1}"], dtype=np.float32) for i in range(len(widths))]
    # biases b{i} are mathematically cancelled by BN mean subtraction; omitted.

    nc = bacc.Bacc("TRN2", target_bir_lowering=False, debug=False, num_devices=NC,
                   num_swdge_queues=NQ)
    build_graph(nc, P, widths)
    nc.compile()

    # input images
    def wimg(W):
        cin, cout = W.shape
        kcs = math.ceil(cin / TILE)
        img = np.zeros((TILE, kcs * cout), dtype=np.float32)
        for kc in range(kcs):
            w = min(TILE, cin - kc * TILE)
            img[:w, kc * cout:(kc + 1) * cout] = W[kc * TILE: kc * TILE + w]
        return img

    def fvec(v):
        cout = v.shape[0]
        hvs = math.ceil(cout / TILE)
        img = np.zeros((TILE, hvs), dtype=np.float32)
        for h in range(hvs):
            hw = min(TILE, cout - h * TILE)
            img[:hw, h] = v[h * TILE: h * TILE + hw]
        return img

    ident = np.eye(TILE, dtype=ml_dtypes.bfloat16)
    identf = np.eye(TILE, dtype=np.float32)
    iota = np.tile(np.arange(TILE, dtype=np.float32), (TILE, 1))
    w5b = wimg(weights[4]).astype(ml_dtypes.bfloat16)

    in_maps = []
    for c in range(NC):
        m = {
            "idx": P.idx_flats[c],
            "dstb": np.ascontiguousarray(
                P.dstw_flats[c][:, 0::2]).astype(ml_dtypes.bfloat16),
            "wb": np.ascontiguousarray(
                P.dstw_flats[c][:, 1::2]).astype(ml_dtypes.bfloat16),
            "maskdinv": P.maskdinv[c],
            "h0T": P.h0T[c],
            "iotab": iota.astype(ml_dtypes.bfloat16),
            "ident": ident,
            "identf": identf,
            "W5b": w5b,
        }
        for li in range(len(widths)):
            m[f"W{li}"] = wimg(weights[li])
            m[f"g{li}"] = fvec(gammas[li])
            m[f"bb{li}"] = fvec(betas[li])
        in_maps.append(m)

    results, times = _run_pjrt(nc, in_maps,
                               trials=int(os.environ.get("GCN_TRIALS", "1")))
    kernel.last_times = times

    out_dev = np.concatenate([results[c]["out"] for c in range(NC)], axis=0)
    out = np.empty((P.N, widths[-1]), dtype=np.float32)
    real = P.orig_of_dev >= 0
    out[P.orig_of_dev[real]] = out_dev[real]
    return out


def measure_floor(trials=6):
    """Null 2-DMA kernel through the same runner: axon dispatch floor."""
    import concourse.bacc as bacc_
    import concourse.tile as tile_
    nc = bacc_.Bacc("TRN2", target_bir_lowering=False, debug=False, num_devices=NC)
    inp = nc.dram_tensor("a", [128, 128], F32, kind="ExternalInput")
    out = nc.dram_tensor("out", [128, 128], F32, kind="ExternalOutput")
    with tile_.TileContext(nc) as tc:
        with tc.tile_pool(name="p", bufs=1) as pool:
            t = pool.tile([128, 128], F32)
            nc.sync.dma_start(t[:], inp[:, :])
            nc.sync.dma_start(out[:, :], t[:])
    nc.compile()
    in_maps = [{"a": np.ones((128, 128), np.float32)} for _ in range(NC)]
    _, times = _run_pjrt(nc, in_maps, trials=trials)
    return times


def _run_pjrt(nc, in_maps, trials=1):
    """Replicates bass2jax.run_bass_via_pjrt but with device-staged inputs and
    wall-clock timing of repeated executions."""
    import time
    import jax
    from jax.sharding import Mesh, PartitionSpec, NamedSharding
    from jax.experimental.shard_map import shard_map
    import concourse.bass2jax as b2j
    import concourse.mybir as mb

    b2j.install_neuronx_cc_hook()
    n_cores = NC
    partition_name = nc.partition_id_tensor.name if nc.partition_id_tensor else None
    in_names, out_names, out_avals, zero_outs = [], [], [], []
    for alloc in nc.m.functions[0].allocations:
        if not isinstance(mb.MemoryLocationSet, type) or not isinstance(alloc, mb.MemoryLocationSet):
            continue
        if not alloc.memorylocations:
            continue
        name = alloc.memorylocations[0].name
        if alloc.kind == "ExternalInput":
            if name != partition_name:
                in_names.append(name)
        elif alloc.kind == "ExternalOutput":
            out_names.append(name)
            shape = tuple(alloc.tensor_shape)
            dtype = mb.dt.np(alloc.dtype)
            out_avals.append(jax.core.ShapedArray(shape, dtype))
            zero_outs.append(np.zeros(shape, dtype))
    n_params = len(in_names)
    n_outs = len(out_avals)
    in_names_all = list(in_names) + list(out_names)
    if partition_name is not None:
        in_names_all.append(partition_name)
    donate = tuple(range(n_params, n_params + n_outs))

    def _body(*args):
        operands = list(args)
        if partition_name is not None:
            operands.append(b2j.partition_id_tensor())
        outs = b2j._bass_exec_p.bind(
            *operands,
            out_avals=tuple(out_avals),
            in_names=tuple(in_names_all),
            out_names=tuple(out_names),
            lowering_input_output_aliases=(),
            sim_require_finite=True,
            sim_require_nnan=True,
            nc=nc,
        )
        return tuple(outs)

    devices = jax.devices()[:n_cores]
    mesh = Mesh(np.asarray(devices), ("core",))
    in_specs = (PartitionSpec("core"),) * (n_params + n_outs)
    out_specs = (PartitionSpec("core"),) * len(out_names)
    sharded = jax.jit(
        shard_map(_body, mesh=mesh, in_specs=in_specs, out_specs=out_specs,
                  check_rep=False),
        donate_argnums=donate, keep_unused=True)
    shd = NamedSharding(mesh, PartitionSpec("core"))
    concat_in = [
        jax.device_put(
            np.concatenate([np.asarray(in_maps[c][nm]) for c in range(n_cores)], axis=0),
            shd)
        for nm in in_names
    ]
    jax.block_until_ready(concat_in)

    times = []
    out_arrs = None
    for t in range(max(1, trials)):
        dev_zeros = [
            jax.device_put(np.zeros((n_cores * z.shape[0], *z.shape[1:]), z.dtype), shd)
            for z in zero_outs
        ]
        jax.block_until_ready(dev_zeros)
        t0 = time.perf_counter()
        out_arrs = sharded(*concat_in, *dev_zeros)
        jax.block_until_ready(out_arrs)
        times.append(time.perf_counter() - t0)

    results = [
        {name: np.asarray(out_arrs[i]).reshape(n_cores, *out_avals[i].shape)[c]
         for i, name in enumerate(out_names)}
        for c in range(n_cores)
    ]
    return results, times
